# revision 15
# baseline (speedup 1.0000x reference)
"""Trainium2 Bass kernel for the dual-stream ViT encoder/decoder (nn_DE_MSFT_50714973831308).

Strategy: pure 8-way data parallelism over batch (2 images per core, no collectives).
On-chip layout: transposed activations [D(part), T(free)], T = 2*197 = 394 tokens.
All matmuls fp16 x fp16 -> fp32 PSUM. Residual stream kept fp32 in SBUF.
LayerNorm stats via ones-matmul column sums; softmax normalization via
ones-matmul column sums of exp scores (keys-on-partitions S^T layout, no transposes).
"""
import numpy as np

import concourse.bacc as bacc
import concourse.mybir as mybir
import concourse.tile as tile
from concourse.bass_utils import run_bass_kernel_spmd

F16 = mybir.dt.float16
F32 = mybir.dt.float32
AF = mybir.ActivationFunctionType

NCORES = 8
BS = 16
BL = BS // NCORES          # batch per core
N = 197                    # tokens per image (196 patches + cls)
T = BL * N                 # token columns per stream per core
DIM = 768
KD = DIM // 128            # 6
HEADS = 12
DH = 64
MLP = 3072
KM = MLP // 128            # 24
DEPTH = 4
NCLS = 1000
SCALE = float(DIM) ** -0.5   # reference scales attention by DIM^-0.5, not DH^-0.5
EXP_SHIFT = -5.0             # constant softmax shift (cancels in normalization)
TCH = [(0, 128), (128, 69)]  # token chunks within one image


# ---------------------------------------------------------------- host packing
class Packer:
    def __init__(self):
        self.w = []      # fp16 chunks
        self.v = []      # fp32 chunks
        self.woff = 0
        self.voff = 0
        self.spec = {}

    def lhsT(self, key, w, mt=128):
        # w [K, F] fp32 -> per m-tile blocks [128, K/128, mt] (partition-major) fp16
        K, F = w.shape
        assert K % 128 == 0 and F % mt == 0
        nmt = F // mt
        arr = np.ascontiguousarray(
            w.reshape(K // 128, 128, nmt, mt).transpose(2, 1, 0, 3),
            dtype=np.float16).ravel()
        self.spec[key] = ("lhsT", self.woff, K, F, mt, nmt)
        self.w.append(arr)
        self.woff += arr.size

    def rhs(self, key, w):
        # w [K, F] -> [128, K/128, F] partition-major fp16
        K, F = w.shape
        arr = np.ascontiguousarray(
            w.reshape(K // 128, 128, F).transpose(1, 0, 2), dtype=np.float16).ravel()
        self.spec[key] = ("rhs", self.woff, K, F)
        self.w.append(arr)
        self.woff += arr.size

    def lnwb(self, key, w, b):
        # rows [w, -w, b] [3, F] fp16
        arr = np.ascontiguousarray(np.stack([w, -w, b]), dtype=np.float16).ravel()
        self.spec[key] = ("lnwb", self.woff, w.shape[0])
        self.w.append(arr)
        self.woff += arr.size

    def vec(self, key, b):
        # bias [F] fp32 -> [psz, nm] partition-major
        F = b.shape[0]
        if F % 128 == 0:
            psz, nm = 128, F // 128
        else:
            psz, nm = 125, F // 125
            assert psz * nm == F
        arr = np.ascontiguousarray(b.reshape(nm, psz).T, dtype=np.float32).ravel()
        self.spec[key] = ("vec", self.voff, psz, nm)
        self.v.append(arr)
        self.voff += arr.size

    def raw3(self, key, a):
        # a [768, t] fp32 -> [128, 6, t] partition-major
        D, t = a.shape
        arr = np.ascontiguousarray(
            a.reshape(D // 128, 128, t).transpose(1, 0, 2), dtype=np.float32).ravel()
        self.spec[key] = ("raw3", self.voff, D, t)
        self.v.append(arr)
        self.voff += arr.size

    def done(self):
        wbuf = np.concatenate(self.w) if self.w else np.zeros(1, np.float16)
        vbuf = np.concatenate(self.v) if self.v else np.zeros(1, np.float32)
        return wbuf, vbuf


def pack_params(p):
    p = {k: np.asarray(v, dtype=np.float32) for k, v in p.items()}
    g = lambda k, *idx: p[k][idx] if idx else p[k]
    pk = Packer()
    for i in range(DEPTH):
        for s in range(2):
            for j in range(2):
                qkv = g('enc_attn_qkv', i, s, j)
                pk.lhsT(f"e{i}{s}{j}.qk", qkv[:, :2 * DIM])
                pk.rhs(f"e{i}{s}{j}.v", qkv[:, 2 * DIM:])
                pk.lhsT(f"e{i}{s}{j}.ow", g('enc_attn_ow', i, s, j))
                pk.vec(f"e{i}{s}{j}.ob", g('enc_attn_ob', i, s, j))
                pk.lnwb(f"e{i}{s}{j}.ln", g('enc_attn_ln_w', i, s, j),
                        g('enc_attn_ln_b', i, s, j))
            pk.lhsT(f"ef{i}{s}.w1", g('enc_ff_w1', i, s))
            pk.vec(f"ef{i}{s}.b1", g('enc_ff_b1', i, s))
            pk.lhsT(f"ef{i}{s}.w2", g('enc_ff_w2', i, s))
            pk.vec(f"ef{i}{s}.b2", g('enc_ff_b2', i, s))
            pk.lnwb(f"ef{i}{s}.ln", g('enc_ff_ln_w', i, s), g('enc_ff_ln_b', i, s))
            qkv = g('dec_self_qkv', i, s)
            pk.lhsT(f"ds{i}{s}.qk", qkv[:, :2 * DIM])
            pk.rhs(f"ds{i}{s}.v", qkv[:, 2 * DIM:])
            pk.lhsT(f"ds{i}{s}.ow", g('dec_self_ow', i, s))
            pk.vec(f"ds{i}{s}.ob", g('dec_self_ob', i, s))
            pk.lnwb(f"ds{i}{s}.ln", g('dec_self_ln_w', i, s), g('dec_self_ln_b', i, s))
            pk.lhsT(f"dc{i}{s}.qw", g('dec_cross_qw', i, s))
            kvw = g('dec_cross_kvw', i, s)
            pk.lhsT(f"dc{i}{s}.kw", kvw[:, :DIM])
            pk.rhs(f"dc{i}{s}.v", kvw[:, DIM:])
            pk.lhsT(f"dc{i}{s}.ow", g('dec_cross_ow', i, s))
            pk.vec(f"dc{i}{s}.ob", g('dec_cross_ob', i, s))
            pk.lnwb(f"dc{i}{s}.ln", g('dec_cross_ln_w', i, s), g('dec_cross_ln_b', i, s))
            pk.lhsT(f"df{i}{s}.w1", g('dec_ff_w1', i, s))
            pk.vec(f"df{i}{s}.b1", g('dec_ff_b1', i, s))
            pk.lhsT(f"df{i}{s}.w2", g('dec_ff_w2', i, s))
            pk.vec(f"df{i}{s}.b2", g('dec_ff_b2', i, s))
            pk.lnwb(f"df{i}{s}.ln", g('dec_ff_ln_w', i, s), g('dec_ff_ln_b', i, s))
    for s, nm in enumerate("AB"):
        pk.lhsT(f"pe{nm}.w", g(f'pe_{nm}_w'))
        pos = g('pos')[0, :N].T                         # [768, 197]
        ext = np.tile(pos, (1, BL))                     # [768, 394]
        peb = g(f'pe_{nm}_b')
        cls = g(f'cls_{nm}')[0, 0]
        for b in range(BL):
            ext[:, b * N] = pos[:, 0] + cls - peb
        pk.raw3(f"pos{nm}", ext)
        pk.vec(f"pe{nm}.b", peb)
    pk.lhsT("head.w", g('head_w'), mt=125)
    pk.vec("head.b", g('head_b'))
    pk.lnwb("head.ln", g('head_ln_w'), g('head_ln_b'))
    return pk


def patchify(img):
    b = img.shape[0]
    g = 224 // 16
    x = img.reshape(b, 3, g, 16, g, 16).transpose(0, 2, 4, 3, 5, 1)
    return x.reshape(b, g * g, 768)


def pack_patches(pt):
    # pt [BL, 196, 768] -> [128, 6, 394] fp16; col b*197 zeroed (cls slot)
    a = np.zeros((768, T), np.float32)
    for b in range(BL):
        a[:, b * N + 1:(b + 1) * N] = pt[b].T
    return np.ascontiguousarray(
        a.reshape(6, 128, T).transpose(1, 0, 2), dtype=np.float16)


# ---------------------------------------------------------------- kernel build
def build(spec):
    nc = bacc.Bacc("TRN2", debug=False, num_devices=NCORES)
    wb = nc.dram_tensor("wbuf", [spec["_wsize"]], F16, kind="ExternalInput").ap()
    vb = nc.dram_tensor("vbuf", [spec["_vsize"]], F32, kind="ExternalInput").ap()
    pA = nc.dram_tensor("pA", [128, KD, T], F16, kind="ExternalInput").ap()
    pB = nc.dram_tensor("pB", [128, KD, T], F16, kind="ExternalInput").ap()
    oenc = nc.dram_tensor("oenc", [BL, NCLS], F32, kind="ExternalOutput").ap()
    odec = nc.dram_tensor("odec", [BL, NCLS], F32, kind="ExternalOutput").ap()

    with tile.TileContext(nc) as tc:
        build_body(nc, tc, spec, wb, vb, pA, pB, oenc, odec)
    nc.compile()
    return nc


def build_body(nc, tc, spec, wb, vb, pA, pB, oenc, odec):
    from contextlib import ExitStack
    ctx = ExitStack()
    p1 = ctx.enter_context(tc.tile_pool(name="persist", bufs=1))
    pxln = ctx.enter_context(tc.tile_pool(name="xln", bufs=4))
    pact = ctx.enter_context(tc.tile_pool(name="act", bufs=2))
    ph = ctx.enter_context(tc.tile_pool(name="hbuf", bufs=1))
    pcast = ctx.enter_context(tc.tile_pool(name="cast", bufs=1))
    pev = ctx.enter_context(tc.tile_pool(name="ev", bufs=3))
    pw = ctx.enter_context(tc.tile_pool(name="w", bufs=2))
    pwv = ctx.enter_context(tc.tile_pool(name="wv", bufs=2))
    pst = ctx.enter_context(tc.tile_pool(name="st", bufs=2))
    pm = ctx.enter_context(tc.tile_pool(name="pm", bufs=3, space="PSUM"))
    pa = ctx.enter_context(tc.tile_pool(name="pa", bufs=3, space="PSUM"))
    ps = ctx.enter_context(tc.tile_pool(name="ps", bufs=2, space="PSUM"))

    ones = p1.tile([128, 1], F16)
    nc.vector.memset(ones[:], 1.0)
    onesr = p1.tile([1, 128], F16)
    nc.vector.memset(onesr[:], 1.0)
    ceps = p1.tile([1, 1], F32)
    nc.vector.memset(ceps[:], 1e-5)
    cshift = p1.tile([128, 1], F32)
    nc.vector.memset(cshift[:], EXP_SHIFT)

    xA = p1.tile([128, KD, T], F32, tag="xA")
    xB = p1.tile([128, KD, T], F32, tag="xB")
    X = {0: xA, 1: xB}

    def wv_ap(key):
        _, off, K, F = spec[key][:4]
        return wb[off:off + K * F].rearrange("(p a b) -> p a b", p=128, a=K // 128)

    def wm_ap(key, m):
        _, off, K, F, mt, nmt = spec[key]
        o = off + m * K * mt
        return wb[o:o + K * mt].rearrange("(p a b) -> p a b", p=128, a=K // 128)

    def ln_ap(key):
        _, off, F = spec[key]
        return wb[off:off + 3 * F].rearrange("(a b) -> a b", a=3)

    def vec_tile(key):
        _, off, psz, nm = spec[key]
        a = vb[off:off + psz * nm].rearrange("(p m) -> p m", p=psz)
        t = pw.tile([psz, nm], F32, tag="bias")
        nc.sync.dma_start(t[:], a)
        return t

    # ---------------- layernorm ----------------
    def ln_stats(x32, t=T):
        xf = pcast.tile([128, KD, T], F16, tag="cast")
        nc.scalar.activation(xf[:, :, 0:t], x32[:, :, 0:t], AF.Copy)
        x2 = pcast.tile([128, KD, T], F16, tag="sq")
        nc.scalar.activation(x2[:, :, 0:t], x32[:, :, 0:t], AF.Square)
        st = ps.tile([33, T], F32, tag="small")
        for kt in range(KD):
            nc.tensor.matmul(st[0:1, 0:t], ones[:, :], xf[:, kt, 0:t],
                             start=(kt == 0), stop=(kt == KD - 1))
        for kt in range(KD):
            nc.tensor.matmul(st[32:33, 0:t], ones[:, :], x2[:, kt, 0:t],
                             start=(kt == 0), stop=(kt == KD - 1))
        m1 = pst.tile([1, T], F32, tag="m1")
        m2 = pst.tile([1, T], F32, tag="m2")
        nc.vector.tensor_scalar_mul(m1[0:1, 0:t], st[0:1, 0:t], 1.0 / DIM)
        nc.vector.tensor_scalar_mul(m2[0:1, 0:t], st[32:33, 0:t], 1.0 / DIM)
        sq = pst.tile([1, T], F32, tag="sqv")
        nc.vector.tensor_mul(sq[0:1, 0:t], m1[0:1, 0:t], m1[0:1, 0:t])
        nc.vector.tensor_sub(m2[0:1, 0:t], m2[0:1, 0:t], sq[0:1, 0:t])
        nc.scalar.activation(sq[0:1, 0:t], m2[0:1, 0:t], AF.Sqrt, bias=ceps[0:1, :])
        nc.vector.reciprocal(m2[0:1, 0:t], sq[0:1, 0:t])   # rs
        rs16 = pst.tile([1, T], F16, tag="rs16")
        nc.vector.tensor_copy(rs16[0:1, 0:t], m2[0:1, 0:t])
        r2 = pst.tile([2, T], F16, tag="r2")
        nc.vector.memset(r2[0:2, 0:t], 1.0)
        nc.vector.tensor_mul(r2[0:1, 0:t], m1[0:1, 0:t], m2[0:1, 0:t])  # c = mean*rs
        return rs16, r2

    def ln_apply(x32, stats, key, t=T):
        # xln = x * outer(w, rs) + (outer(-w, c) + outer(b, 1))
        rs16, r2 = stats
        _, off, F = spec[key]
        lwA = pw.tile([1, DIM], F16, tag="lnA")
        nc.sync.dma_start(lwA[:], wb[off:off + F].rearrange("(a b) -> a b", a=1))
        lwB = pw.tile([2, DIM], F16, tag="lnB")
        nc.sync.dma_start(lwB[:], wb[off + F:off + 3 * F].rearrange("(a b) -> a b", a=2))
        out = pxln.tile([128, KD, T], F16, tag="xln")
        for mt in range(KD):
            bc1 = pa.tile([128, 2 * N], F32, tag="psA")
            bc2 = pa.tile([128, 2 * N], F32, tag="psA")
            sl = slice(mt * 128, (mt + 1) * 128)
            nc.tensor.matmul(bc1[:, 0:t], lwA[0:1, sl], rs16[0:1, 0:t],
                             start=True, stop=True)
            nc.tensor.matmul(bc2[:, 0:t], lwB[0:2, sl], r2[0:2, 0:t],
                             start=True, stop=True)
            nc.vector.tensor_mul(out[:, mt, 0:t], x32[:, mt, 0:t], bc1[:, 0:t])
            nc.vector.tensor_add(out[:, mt, 0:t], out[:, mt, 0:t], bc2[:, 0:t])
        return out

    def ln_full(x32, key, t=T):
        return ln_apply(x32, ln_stats(x32, t), key, t)

    # ---------------- linear (lhsT weights, transposed activations) ----------
    def linear(xin, key, out_tag=None, bias=None, act=None, residual=None,
               kts=KD, t=T):
        _, off, K, F, mt, nmt = spec[key]
        assert K == kts * 128 and mt == 128
        bt = vec_tile(bias) if bias else None
        out = None
        if residual is None:
            pool = ph if out_tag == "h16" else pact
            out = pool.tile([128, nmt, T], F16, tag=out_tag, name=out_tag)
        for m in range(nmt):
            wt = pw.tile([128, kts, 128], F16, tag=f"w{kts}")
            nc.sync.dma_start(wt[:], wm_ap(key, m))
            psum = pm.tile([128, T], F32, tag="mm")
            for kt in range(kts):
                nc.tensor.matmul(psum[:, 0:t], wt[:, kt, :], xin[:, kt, 0:t],
                                 start=(kt == 0), stop=(kt == kts - 1))
            if residual is not None:
                tmp = pev.tile([128, T], F16, tag="ev")
                if bt is not None:
                    nc.scalar.activation(tmp[:, 0:t], psum[:, 0:t], AF.Identity,
                                         bias=bt[:, m:m + 1])
                else:
                    nc.scalar.activation(tmp[:, 0:t], psum[:, 0:t], AF.Copy)
                nc.vector.tensor_add(residual[:, m, 0:t], residual[:, m, 0:t],
                                     tmp[:, 0:t])
            elif act is not None:
                nc.scalar.activation(out[:, m, 0:t], psum[:, 0:t], act,
                                     bias=bt[:, m:m + 1])
            elif bt is not None:
                nc.scalar.activation(out[:, m, 0:t], psum[:, 0:t], AF.Identity,
                                     bias=bt[:, m:m + 1])
            else:
                nc.scalar.activation(out[:, m, 0:t], psum[:, 0:t], AF.Copy)
        return out

    def v_producer(xin, key):
        # token-major V: [128(token-in-chunk), 4(chunk=b*2+c), 768]
        _, off, K, F = spec[key][:4]
        wt = pwv.tile([128, KD, F], F16, tag="wv")
        nc.sync.dma_start(wt[:], wv_ap(key))
        v16 = pact.tile([128, 2 * BL, F], F16, tag="v16")
        for b in range(BL):
            for c, (o, sz) in enumerate(TCH):
                col = b * N + o
                for nch in range(2):
                    n0 = nch * 384
                    psum = pm.tile([128, T], F32, tag="mm")
                    for kt in range(KD):
                        nc.tensor.matmul(psum[0:sz, 0:384],
                                         xin[:, kt, col:col + sz],
                                         wt[:, kt, n0:n0 + 384],
                                         start=(kt == 0), stop=(kt == KD - 1))
                    nc.scalar.activation(v16[0:sz, b * 2 + c, n0:n0 + 384],
                                         psum[0:sz, 0:384], AF.Copy)
        return v16

    def attn_core(q16, k16, koff, v16):
        o16 = pact.tile([128, KD, T], F16, tag="o16")
        for b in range(BL):
            qcol = slice(b * N, (b + 1) * N)
            for h in range(HEADS):
                prow = slice((h % 2) * 64, (h % 2) * 64 + 64)
                qs = q16[prow, h // 2, qcol]
                ks = k16[prow, koff + h // 2, qcol]
                sT = pa.tile([128, 2 * N], F32, tag="psA")
                for c, (o, sz) in enumerate(TCH):
                    nc.tensor.matmul(sT[0:sz, c * N:(c + 1) * N], ks[:, o:o + sz], qs,
                                     start=True, stop=True)
                eS = pev.tile([128, 2, N], F16, tag="eS")
                for c, (o, sz) in enumerate(TCH):
                    nc.scalar.activation(eS[0:sz, c, :], sT[0:sz, c * N:(c + 1) * N],
                                         AF.Exp, scale=SCALE, bias=cshift[0:sz, :])
                z = ps.tile([1, T], F32, tag="small")
                for c, (o, sz) in enumerate(TCH):
                    nc.tensor.matmul(z[0:1, 0:N], ones[0:sz, :], eS[0:sz, c, :],
                                     start=(c == 0), stop=(c == 1))
                rz32 = pst.tile([1, T], F32, tag="rz32")
                nc.vector.reciprocal(rz32[0:1, 0:N], z[0:1, 0:N])
                rz16 = pst.tile([1, T], F16, tag="rz16")
                nc.vector.tensor_copy(rz16[0:1, 0:N], rz32[0:1, 0:N])
                obrz = pm.tile([128, T], F32, tag="mm")
                for c, (o, sz) in enumerate(TCH):
                    nc.tensor.matmul(obrz[0:64, 0:N],
                                     v16[0:sz, b * 2 + c, h * 64:(h + 1) * 64],
                                     eS[0:sz, c, :], start=(c == 0), stop=(c == 1))
                nc.tensor.matmul(obrz[64:128, 0:N], onesr[0:1, 0:64],
                                 rz16[0:1, 0:N], start=True, stop=True)
                osl = o16[prow, h // 2, qcol]
                nc.scalar.activation(osl, obrz[0:64, 0:N], AF.Copy)
                nc.vector.tensor_mul(osl, osl, obrz[64:128, 0:N])
        return o16

    def self_attn(x32, pref):
        xln = ln_full(x32, f"{pref}.ln")
        qk16 = linear(xln, f"{pref}.qk", out_tag="qk16")
        v16 = v_producer(xln, f"{pref}.v")
        o16 = attn_core(qk16, qk16, KD, v16)
        linear(o16, f"{pref}.ow", bias=f"{pref}.ob", residual=x32)

    def ff(x32, pref):
        xln = ln_full(x32, f"{pref}.ln")
        h16 = linear(xln, f"{pref}.w1", out_tag="h16", bias=f"{pref}.b1", act=AF.Gelu)
        linear(h16, f"{pref}.w2", bias=f"{pref}.b2", residual=x32, kts=KM)

    # ---------------- model ----------------
    with nc.named_scope("embed"):
        for s in range(2):
            nm = "AB"[s]
            x32 = X[s]
            _, off, D, t = spec[f"pos{nm}"]
            nc.sync.dma_start(x32[:], vb[off:off + D * t].rearrange(
                "(p a b) -> p a b", p=128, a=KD))
            pa16 = pact.tile([128, KD, T], F16, tag="o16")
            nc.sync.dma_start(pa16[:], [pA, pB][s])
            bt = vec_tile(f"pe{nm}.b")
            for m in range(KD):
                wt = pw.tile([128, KD, 128], F16, tag="w6")
                nc.sync.dma_start(wt[:], wm_ap(f"pe{nm}.w", m))
                psum = pm.tile([128, T], F32, tag="mm")
                for kt in range(KD):
                    nc.tensor.matmul(psum[:, :], wt[:, kt, :], pa16[:, kt, :],
                                     start=(kt == 0), stop=(kt == KD - 1))
                tmp = pev.tile([128, T], F16, tag="ev")
                nc.scalar.activation(tmp[:, :], psum[:, :], AF.Identity,
                                     bias=bt[:, m:m + 1])
                nc.vector.tensor_add(x32[:, m, :], x32[:, m, :], tmp[:, :])

    for i in range(DEPTH):
        for s in range(2):
            with nc.named_scope(f"enc{i}s{s}"):
                for j in range(2):
                    self_attn(X[s], f"e{i}{s}{j}")
                ff(X[s], f"ef{i}{s}")

    clsE = p1.tile([128, KD, 2], F32, tag="clsE")
    for b in range(BL):
        nc.vector.tensor_add(clsE[:, :, b:b + 1], xA[:, :, b * N:b * N + 1],
                             xB[:, :, b * N:b * N + 1])

    def head(cls32, out_dram):
        xln = ln_full(cls32, "head.ln", t=2)
        bt = vec_tile("head.b")
        for m in range(8):
            wt = pw.tile([128, KD, 125], F16, tag="wh")
            nc.sync.dma_start(wt[:], wm_ap("head.w", m))
            psum = pm.tile([128, T], F32, tag="mm")
            for kt in range(KD):
                nc.tensor.matmul(psum[0:125, 0:2], wt[:, kt, :], xln[:, kt, 0:2],
                                 start=(kt == 0), stop=(kt == KD - 1))
            hs = pst.tile([125, 2], F32, tag="hsb")
            nc.scalar.activation(hs[:, :], psum[0:125, 0:2], AF.Identity,
                                 bias=bt[:, m:m + 1])
            nc.sync.dma_start(
                out_dram[:, m * 125:(m + 1) * 125].rearrange("b m -> m b"), hs[:, :])

    with nc.named_scope("head_enc"):
        head(clsE, oenc)

    for i in range(DEPTH):
        with nc.named_scope(f"dec{i}"):
            for s in range(2):
                self_attn(X[s], f"ds{i}{s}")
            stA = ln_stats(xA)
            lnA_xA = ln_apply(xA, stA, f"dc{i}0.ln")
            lnB_xA = ln_apply(xA, stA, f"dc{i}1.ln")
            stB = ln_stats(xB)
            lnA_xB = ln_apply(xB, stB, f"dc{i}0.ln")
            lnB_xB = ln_apply(xB, stB, f"dc{i}1.ln")
            for s, (xlnq, xlnkv) in enumerate([(lnA_xA, lnA_xB), (lnB_xB, lnB_xA)]):
                pref = f"dc{i}{s}"
                q16 = linear(xlnq, f"{pref}.qw", out_tag="q16")
                k16 = linear(xlnkv, f"{pref}.kw", out_tag="k16")
                v16 = v_producer(xlnkv, f"{pref}.v")
                o16 = attn_core(q16, k16, 0, v16)
                linear(o16, f"{pref}.ow", bias=f"{pref}.ob", residual=X[s])
            for s in range(2):
                ff(X[s], f"df{i}{s}")

    clsD = p1.tile([128, KD, 2], F32, tag="clsD")
    for b in range(BL):
        nc.vector.tensor_add(clsD[:, :, b:b + 1], xA[:, :, b * N:b * N + 1],
                             xB[:, :, b * N:b * N + 1])
    with nc.named_scope("head_dec"):
        head(clsD, odec)

    ctx.close()


# ---------------------------------------------------------------- entry point
_CACHE = {}


def _prepare(params):
    pk = pack_params(params)
    wbuf, vbuf = pk.done()
    spec = dict(pk.spec)
    spec["_wsize"] = int(wbuf.size)
    spec["_vsize"] = int(vbuf.size)
    nc = build(spec)
    return nc, wbuf, vbuf


def make_in_maps(fea_A, fea_B, wbuf, vbuf):
    ptA = patchify(np.asarray(fea_A, dtype=np.float32))
    ptB = patchify(np.asarray(fea_B, dtype=np.float32))
    in_maps = []
    for c in range(NCORES):
        sl = slice(c * BL, (c + 1) * BL)
        in_maps.append({
            "wbuf": wbuf, "vbuf": vbuf,
            "pA": pack_patches(ptA[sl]),
            "pB": pack_patches(ptB[sl]),
        })
    return in_maps


def kernel(fea_A, fea_B, params):
    if "k" not in _CACHE:
        _CACHE["k"] = _prepare(params)
    nc, wbuf, vbuf = _CACHE["k"]
    in_maps = make_in_maps(fea_A, fea_B, wbuf, vbuf)
    res = run_bass_kernel_spmd(nc, in_maps, core_ids=list(range(NCORES)))
    enc = np.concatenate([res.results[c]["oenc"] for c in range(NCORES)], axis=0)
    dec = np.concatenate([res.results[c]["odec"] for c in range(NCORES)], axis=0)
    return enc, dec
